# revision 20
# baseline (speedup 1.0000x reference)
"""Trainium2 Bass kernel for nn_MembraneLayer: h = x @ w followed by a
double first-order recurrence over time, producing (syn_rec, mem_rec).

Sharding: data-parallel over batch. 8 cores x 64 batches each.

Per-core device layout (all hardcoded):
  xt    [700, 6400]  f32   x transposed to [C, b*T+t] (host-prepped)
  w_    [700, 512]   f32
  acoef [4, 128, 500] f32  alpha per (d_tile, partition), 0 at t=0 slots
  bcoef [4, 128, 500] f32  beta likewise
  omb   [128, 4]     f32   (1 - beta) per (partition, d_tile)
  syn   [512, 6400]  f32   out: syn_rec in [D, b*T+t] layout
  mem   [512, 6400]  f32   out: mem_rec[t+1] written shifted; t=0 zeroed on host

Per b-group (5 batches = 500 time-slots; bank-sized), per d_tile (128 rows
of D): the matmul accumulates h into PSUM at slot t+1 (shifted write), the
t=0 columns are memset to zero, and a DVE tensor_tensor_scan computes
syn_rec = alpha*prev + h directly from PSUM. A second scan over
(1-beta)*syn_rec yields mem_rec shifted by one step.
"""

import os

import numpy as np

import concourse.bass as bass
import concourse.tile as tile
from concourse import bacc, mybir
from concourse import bass_utils

B, T, C, D = 512, 100, 700, 512
NCORES = 8
BC = B // NCORES  # 64
NG = 16  # uniform groups of 4 batches (400 slots; one PSUM bank each)
NQ = 4  # quartets: 4 groups share one x staging load
KT = [(k * 128, min(128, C - k * 128)) for k in range(6)]
F32 = mybir.dt.float32
BF16 = mybir.dt.bfloat16

# "fp32": plain fp32 matmuls (4 cyc/row).  "split3": x and w split into
# bf16 hi + lo on host; h = xh@wh + xh@wl + xl@wh (3 bf16 passes).
# "fp16x2": x split into fp16 hi + lo, w rounded to fp16; h = (xh+xl)@w16
# (2 fp16 passes; error dominated by w rounding ~2^-12).
MODE = os.environ.get("MEMBRANE_MM_MODE", "fp16x2")

LAST_RESULT = None
_cache = {}


def _build(mode):
    key = ("nc", mode)
    if key in _cache:
        return _cache[key]
    nc = bacc.Bacc("TRN2", target_bir_lowering=False, debug=False)

    FP16 = mybir.dt.float16
    if mode == "fp32":
        xt_d = [nc.dram_tensor("xt", [C, BC * T], F32, kind="ExternalInput").ap()]
        w_d = [nc.dram_tensor("w_", [C, D], F32, kind="ExternalInput").ap()]
    elif mode == "fp16x2":
        xt_d = [
            nc.dram_tensor("xth", [C, BC * T], FP16, kind="ExternalInput").ap(),
            nc.dram_tensor("xtl", [C, BC * T], FP16, kind="ExternalInput").ap(),
        ]
        w_d = [nc.dram_tensor("w16", [C, D], FP16, kind="ExternalInput").ap()]
    else:
        xt_d = [
            nc.dram_tensor("xth", [C, BC * T], BF16, kind="ExternalInput").ap(),
            nc.dram_tensor("xtl", [C, BC * T], BF16, kind="ExternalInput").ap(),
        ]
        w_d = [
            nc.dram_tensor("wh", [C, D], BF16, kind="ExternalInput").ap(),
            nc.dram_tensor("wl", [C, D], BF16, kind="ExternalInput").ap(),
        ]
    ac_d = nc.dram_tensor("acoef", [4, 128, 500], F32, kind="ExternalInput").ap()
    bc_d = nc.dram_tensor("bcoef", [4, 128, 500], F32, kind="ExternalInput").ap()
    syn_d = nc.dram_tensor("syn", [D, BC * T], F32, kind="ExternalOutput").ap()
    mem_d = nc.dram_tensor("mem", [D, BC * T], F32, kind="ExternalOutput").ap()

    wdt = {"fp32": F32, "fp16x2": FP16}.get(mode, BF16)
    nstream = len(xt_d)  # 1 (fp32) or 2 (hi, lo)

    with tile.TileContext(nc) as tc:
        from contextlib import ExitStack

        with ExitStack() as ctx:
            cpool = ctx.enter_context(tc.tile_pool(name="consts", bufs=1))
            # weights: per k-tile, per stream
            w_tiles = []
            for s in range(len(w_d)):
                row = []
                for k, (r0, rk) in enumerate(KT):
                    wt = cpool.tile([128, D], wdt, name=f"w{s}_{k}", tag=f"w{s}_{k}")
                    nc.sync.dma_start(wt[:rk, :], w_d[s][r0 : r0 + rk, :])
                    row.append(wt)
                w_tiles.append(row)
            # coef loads on the gpsimd (SWDGE) queue so the Sync queue leads
            # with the first x tiles (shorter PE lead-in)
            ac_t, bc_t = [], []
            for i in range(4):
                a = cpool.tile([128, 500], F32, name=f"ac{i}", tag=f"ac{i}")
                nc.gpsimd.dma_start(a[:], ac_d[i])
                ac_t.append(a)
                b = cpool.tile([128, 500], F32, name=f"bc{i}", tag=f"bc{i}")
                nc.gpsimd.dma_start(b[:], bc_d[i])
                bc_t.append(b)

            xp = ctx.enter_context(tc.tile_pool(name="xp", bufs=2))
            pp = ctx.enter_context(tc.tile_pool(name="pp", bufs=8, space="PSUM"))
            syp = ctx.enter_context(tc.tile_pool(name="syp", bufs=3))
            vp = ctx.enter_context(tc.tile_pool(name="vp", bufs=3))

            # weight selections, in accumulation order: (x_stream, w_stream, k)
            if mode == "fp32":
                passes = [(0, 0, k) for k in range(6)]
            elif mode == "fp16x2":
                passes = [(0, 0, k) for k in range(6)] + [(1, 0, k) for k in range(6)]
            else:
                passes = (
                    [(0, 0, k) for k in range(6)]
                    + [(0, 1, k) for k in range(6)]
                    + [(1, 0, k) for k in range(6)]
                )

            for q in range(NQ):
                qcols = 1600  # 4 groups x 400 slots
                qc0 = q * qcols
                xts = []
                for s in range(nstream):
                    row = []
                    for k, (r0, rk) in enumerate(KT):
                        t_ = xp.tile(
                            [128, qcols], wdt, tag=f"x{s}_{k}", name=f"x{s}_{k}_{q}"
                        )
                        nc.sync.dma_start(
                            t_[:rk, :], xt_d[s][r0 : r0 + rk, qc0 : qc0 + qcols]
                        )
                        row.append(t_)
                    xts.append(row)

                for di in range(4):
                    dsl = slice(di * 128, (di + 1) * 128)
                    pss = [
                        pp.tile([128, 512], F32, tag="ps", name=f"ps_{q}_{di}_{g}")
                        for g in range(4)
                    ]
                    # one weight tile feeds 4 consecutive matmuls (4 banks)
                    for pi, (sx, sw, k) in enumerate(passes):
                        r0, rk = KT[k]
                        lhsT = w_tiles[sw][k][:rk, dsl]
                        for g in range(4):
                            rhs3 = xts[sx][k][:rk, g * 400 : (g + 1) * 400].rearrange(
                                "p (b t) -> p b t", t=100
                            )[:, :, 0:99]
                            out3 = pss[g][:, 0:400].rearrange(
                                "p (b t) -> p b t", t=100
                            )[:, :, 1:100]
                            nc.tensor.matmul(
                                out3,
                                lhsT,
                                rhs3,
                                start=(pi == 0),
                                stop=(pi == len(passes) - 1),
                            )

                    for g in range(4):
                        b0 = q * 16 + g * 4  # first batch of this group
                        c0 = b0 * 100
                        ps = pss[g]
                        z2 = ps[:, 0:400].rearrange("p (b t) -> p b t", t=100)[
                            :, :, 0:1
                        ]
                        nc.vector.memset(z2, 0.0)

                        syn = syp.tile([128, 400], F32, tag="syn", name=f"sy_{q}_{di}_{g}")
                        nc.vector.tensor_tensor_scan(
                            syn[:],
                            ac_t[di][:, :400],
                            ps[:, :400],
                            0.0,
                            mybir.AluOpType.mult,
                            mybir.AluOpType.add,
                        )
                        nc.scalar.dma_start(syn_d[dsl, c0 : c0 + 400], syn[:])
                        # scan u = mem/(1-beta) directly from syn_rec; the
                        # (1-beta) scale is applied on the host during unshard
                        v = vp.tile([128, 400], F32, tag="v", name=f"v_{q}_{di}_{g}")
                        nc.vector.tensor_tensor_scan(
                            v[:],
                            bc_t[di][:, :400],
                            syn[:],
                            0.0,
                            mybir.AluOpType.mult,
                            mybir.AluOpType.add,
                        )
                        # mem_rec[t+1] = v[t]: flat shifted store; each group's
                        # trailing junk lands on the next t=0 slot, zeroed on host
                        nw = 400 if c0 + 401 <= BC * T else 399
                        nc.scalar.dma_start(
                            mem_d[dsl, c0 + 1 : c0 + 1 + nw], v[:, :nw]
                        )

    nc.compile()
    _cache[key] = nc
    return nc


def kernel(inputs, w, alpha, beta):
    global LAST_RESULT
    inputs = np.asarray(inputs, dtype=np.float32)
    w = np.asarray(w, dtype=np.float32)
    alpha = np.asarray(alpha, dtype=np.float32).reshape(-1)
    beta = np.asarray(beta, dtype=np.float32).reshape(-1)

    nc = _build(MODE)

    # constants shared by all cores
    acoef = np.broadcast_to(
        alpha.reshape(4, 128, 1), (4, 128, 500)
    ).astype(np.float32).copy()
    acoef[:, :, ::100] = 0.0
    bcoef = np.broadcast_to(
        beta.reshape(4, 128, 1), (4, 128, 500)
    ).astype(np.float32).copy()
    bcoef[:, :, ::100] = 0.0
    omb_col = (1.0 - beta).reshape(D, 1)  # host-side scale for mem

    if MODE == "split3":
        import ml_dtypes

        wh = w.astype(ml_dtypes.bfloat16)
        wl = (w - wh.astype(np.float32)).astype(ml_dtypes.bfloat16)

    in_maps = []
    for c in range(NCORES):
        xc = inputs[c * BC : (c + 1) * BC]  # [64, 100, 700]
        xt = np.ascontiguousarray(xc.reshape(BC * T, C).T)  # [700, 6400]
        m = {
            "acoef": acoef,
            "bcoef": bcoef,
        }
        if MODE == "fp32":
            m["xt"] = xt
            m["w_"] = w
        elif MODE == "fp16x2":
            xth = xt.astype(np.float16)
            m["xth"] = xth
            m["xtl"] = (xt - xth.astype(np.float32)).astype(np.float16)
            m["w16"] = w.astype(np.float16)
        else:
            xth = xt.astype(ml_dtypes.bfloat16)
            m["xth"] = xth
            m["xtl"] = (xt - xth.astype(np.float32)).astype(ml_dtypes.bfloat16)
            m["wh"] = wh
            m["wl"] = wl
        in_maps.append(m)

    run_kwargs = {}
    if os.environ.get("MEMBRANE_TRACE_DIR"):
        run_kwargs["tmpdir"] = os.environ["MEMBRANE_TRACE_DIR"]
    res = bass_utils.run_bass_kernel_spmd(
        nc, in_maps, core_ids=list(range(NCORES)), **run_kwargs
    )
    LAST_RESULT = res

    syn_full = np.empty((B, T, D), dtype=np.float32)
    mem_full = np.empty((B, T, D), dtype=np.float32)
    for c in range(NCORES):
        r = res.results[c]
        syn_full[c * BC : (c + 1) * BC] = (
            r["syn"].reshape(D, BC, T).transpose(1, 2, 0)
        )
        mem_full[c * BC : (c + 1) * BC] = (
            (r["mem"] * omb_col).reshape(D, BC, T).transpose(1, 2, 0)
        )
    syn_full[:, 0, :] = 0.0
    mem_full[:, 0, :] = 0.0
    return (syn_full, mem_full)


# revision 25
# speedup vs baseline: 1.0290x; 1.0290x over previous
"""Trainium2 Bass kernel for nn_MembraneLayer: h = x @ w followed by a
double first-order recurrence over time, producing (syn_rec, mem_rec).

Sharding: data-parallel over batch. 8 cores x 64 batches each.

Per-core device layout (all hardcoded):
  xt    [700, 6400]  f32   x transposed to [C, b*T+t] (host-prepped)
  w_    [700, 512]   f32
  acoef [4, 128, 500] f32  alpha per (d_tile, partition), 0 at t=0 slots
  bcoef [4, 128, 500] f32  beta likewise
  omb   [128, 4]     f32   (1 - beta) per (partition, d_tile)
  syn   [512, 6400]  f32   out: syn_rec in [D, b*T+t] layout
  mem   [512, 6400]  f32   out: mem_rec[t+1] written shifted; t=0 zeroed on host

Per b-group (5 batches = 500 time-slots; bank-sized), per d_tile (128 rows
of D): the matmul accumulates h into PSUM at slot t+1 (shifted write), the
t=0 columns are memset to zero, and a DVE tensor_tensor_scan computes
syn_rec = alpha*prev + h directly from PSUM. A second scan over
(1-beta)*syn_rec yields mem_rec shifted by one step.
"""

import os

import numpy as np

import concourse.bass as bass
import concourse.tile as tile
from concourse import bacc, mybir
from concourse import bass_utils

B, T, C, D = 512, 100, 700, 512
NCORES = 8
BC = B // NCORES  # 64
NG = 16  # uniform groups of 4 batches (400 slots; one PSUM bank each)
NQ = 4  # quartets: 4 groups share one x staging load
KT = [(k * 128, min(128, C - k * 128)) for k in range(6)]
F32 = mybir.dt.float32
BF16 = mybir.dt.bfloat16

# "fp32": plain fp32 matmuls (4 cyc/row).  "split3": x and w split into
# bf16 hi + lo on host; h = xh@wh + xh@wl + xl@wh (3 bf16 passes).
# "fp16x2": x split into fp16 hi + lo, w rounded to fp16; h = (xh+xl)@w16
# (2 fp16 passes; error dominated by w rounding ~2^-12).
MODE = os.environ.get("MEMBRANE_MM_MODE", "fp16x2")

LAST_RESULT = None
_cache = {}


def _build(mode):
    key = ("nc", mode)
    if key in _cache:
        return _cache[key]
    nc = bacc.Bacc("TRN2", target_bir_lowering=False, debug=False)

    FP16 = mybir.dt.float16
    if mode == "fp32":
        xt_d = [nc.dram_tensor("xt", [C, BC * T], F32, kind="ExternalInput").ap()]
        w_d = [nc.dram_tensor("w_", [C, D], F32, kind="ExternalInput").ap()]
    elif mode == "fp16x2":
        xt_d = [
            nc.dram_tensor("xth", [C, BC * T], FP16, kind="ExternalInput").ap(),
            nc.dram_tensor("xtl", [C, BC * T], FP16, kind="ExternalInput").ap(),
        ]
        w_d = [nc.dram_tensor("w16", [C, D], FP16, kind="ExternalInput").ap()]
    else:
        xt_d = [
            nc.dram_tensor("xth", [C, BC * T], BF16, kind="ExternalInput").ap(),
            nc.dram_tensor("xtl", [C, BC * T], BF16, kind="ExternalInput").ap(),
        ]
        w_d = [
            nc.dram_tensor("wh", [C, D], BF16, kind="ExternalInput").ap(),
            nc.dram_tensor("wl", [C, D], BF16, kind="ExternalInput").ap(),
        ]
    ac_d = nc.dram_tensor("acoef", [4, 128, 500], F32, kind="ExternalInput").ap()
    bc_d = nc.dram_tensor("bcoef", [4, 128, 1600], F32, kind="ExternalInput").ap()
    syn_d = nc.dram_tensor("syn", [D, BC * T], F32, kind="ExternalOutput").ap()
    mem_d = nc.dram_tensor("mem", [D, BC * T], F32, kind="ExternalOutput").ap()

    wdt = {"fp32": F32, "fp16x2": FP16}.get(mode, BF16)
    nstream = len(xt_d)  # 1 (fp32) or 2 (hi, lo)

    with tile.TileContext(nc) as tc:
        from contextlib import ExitStack

        with ExitStack() as ctx:
            cpool = ctx.enter_context(tc.tile_pool(name="consts", bufs=1))
            # weights: per k-tile, per stream
            w_tiles = []
            for s in range(len(w_d)):
                row = []
                for k, (r0, rk) in enumerate(KT):
                    wt = cpool.tile([128, D], wdt, name=f"w{s}_{k}", tag=f"w{s}_{k}")
                    nc.sync.dma_start(wt[:rk, :], w_d[s][r0 : r0 + rk, :])
                    row.append(wt)
                w_tiles.append(row)
            # coef loads on the gpsimd (SWDGE) queue so the Sync queue leads
            # with the first x tiles (shorter PE lead-in)
            ac_t, bc_t = [], []
            for i in range(4):
                a = cpool.tile([128, 500], F32, name=f"ac{i}", tag=f"ac{i}")
                nc.gpsimd.dma_start(a[:], ac_d[i])
                ac_t.append(a)
                b = cpool.tile([128, 1600], F32, name=f"bc{i}", tag=f"bc{i}")
                nc.gpsimd.dma_start(b[:], bc_d[i])
                bc_t.append(b)

            xp = ctx.enter_context(tc.tile_pool(name="xp", bufs=2))
            pp = ctx.enter_context(tc.tile_pool(name="pp", bufs=8, space="PSUM"))
            syp = ctx.enter_context(tc.tile_pool(name="syp", bufs=2))
            vp = ctx.enter_context(tc.tile_pool(name="vp", bufs=2))

            # weight selections, in accumulation order: (x_stream, w_stream, k)
            if mode == "fp32":
                passes = [(0, 0, k) for k in range(6)]
            elif mode == "fp16x2":
                passes = [(0, 0, k) for k in range(6)] + [(1, 0, k) for k in range(6)]
            else:
                passes = (
                    [(0, 0, k) for k in range(6)]
                    + [(0, 1, k) for k in range(6)]
                    + [(1, 0, k) for k in range(6)]
                )

            for q in range(NQ):
                qcols = 1600  # 4 groups x 400 slots
                qc0 = q * qcols
                xts = []
                for s in range(nstream):
                    row = []
                    for k, (r0, rk) in enumerate(KT):
                        t_ = xp.tile(
                            [128, qcols], wdt, tag=f"x{s}_{k}", name=f"x{s}_{k}_{q}"
                        )
                        nc.sync.dma_start(
                            t_[:rk, :], xt_d[s][r0 : r0 + rk, qc0 : qc0 + qcols]
                        )
                        row.append(t_)
                    xts.append(row)

                for di in range(4):
                    dsl = slice(di * 128, (di + 1) * 128)
                    pss = [
                        pp.tile([128, 512], F32, tag="ps", name=f"ps_{q}_{di}_{g}")
                        for g in range(4)
                    ]
                    # one weight tile feeds 4 consecutive matmuls (4 banks)
                    for pi, (sx, sw, k) in enumerate(passes):
                        r0, rk = KT[k]
                        lhsT = w_tiles[sw][k][:rk, dsl]
                        for g in range(4):
                            rhs3 = xts[sx][k][:rk, g * 400 : (g + 1) * 400].rearrange(
                                "p (b t) -> p b t", t=100
                            )[:, :, 0:99]
                            out3 = pss[g][:, 0:400].rearrange(
                                "p (b t) -> p b t", t=100
                            )[:, :, 1:100]
                            nc.tensor.matmul(
                                out3,
                                lhsT,
                                rhs3,
                                start=(pi == 0),
                                stop=(pi == len(passes) - 1),
                            )

                    # 4 syn scans write quarters of one merged tile; then one
                    # mem scan + one store per output for the whole quartet
                    syn = syp.tile([128, 1600], F32, tag="syn", name=f"sy_{q}_{di}")
                    for g in range(4):
                        ps = pss[g]
                        z2 = ps[:, 0:400].rearrange("p (b t) -> p b t", t=100)[
                            :, :, 0:1
                        ]
                        nc.vector.memset(z2, 0.0)
                        nc.vector.tensor_tensor_scan(
                            syn[:, g * 400 : (g + 1) * 400],
                            ac_t[di][:, :400],
                            ps[:, :400],
                            0.0,
                            mybir.AluOpType.mult,
                            mybir.AluOpType.add,
                        )
                    nc.sync.dma_start(syn_d[dsl, qc0 : qc0 + 1600], syn[:])
                    # scan u = mem/(1-beta) directly from syn_rec; the (1-beta)
                    # scale is applied on the host during unshard
                    v = vp.tile([128, 1600], F32, tag="v", name=f"v_{q}_{di}")
                    nc.vector.tensor_tensor_scan(
                        v[:],
                        bc_t[di][:],
                        syn[:],
                        0.0,
                        mybir.AluOpType.mult,
                        mybir.AluOpType.add,
                    )
                    # mem_rec[t+1] = v[t]: flat shifted store; per-series junk
                    # lands on the next t=0 slot, zeroed on host
                    nc.scalar.dma_start(mem_d[dsl, qc0 + 1 : qc0 + 1600], v[:, :1599])

    nc.compile()
    _cache[key] = nc
    return nc


def kernel(inputs, w, alpha, beta):
    global LAST_RESULT
    inputs = np.asarray(inputs, dtype=np.float32)
    w = np.asarray(w, dtype=np.float32)
    alpha = np.asarray(alpha, dtype=np.float32).reshape(-1)
    beta = np.asarray(beta, dtype=np.float32).reshape(-1)

    nc = _build(MODE)

    # constants shared by all cores
    acoef = np.broadcast_to(
        alpha.reshape(4, 128, 1), (4, 128, 500)
    ).astype(np.float32).copy()
    acoef[:, :, ::100] = 0.0
    bcoef = np.broadcast_to(
        beta.reshape(4, 128, 1), (4, 128, 1600)
    ).astype(np.float32).copy()
    bcoef[:, :, ::100] = 0.0
    omb_col = (1.0 - beta).reshape(D, 1)  # host-side scale for mem

    if MODE == "split3":
        import ml_dtypes

        wh = w.astype(ml_dtypes.bfloat16)
        wl = (w - wh.astype(np.float32)).astype(ml_dtypes.bfloat16)

    in_maps = []
    for c in range(NCORES):
        xc = inputs[c * BC : (c + 1) * BC]  # [64, 100, 700]
        xt = np.ascontiguousarray(xc.reshape(BC * T, C).T)  # [700, 6400]
        m = {
            "acoef": acoef,
            "bcoef": bcoef,
        }
        if MODE == "fp32":
            m["xt"] = xt
            m["w_"] = w
        elif MODE == "fp16x2":
            xth = xt.astype(np.float16)
            m["xth"] = xth
            m["xtl"] = (xt - xth.astype(np.float32)).astype(np.float16)
            m["w16"] = w.astype(np.float16)
        else:
            xth = xt.astype(ml_dtypes.bfloat16)
            m["xth"] = xth
            m["xtl"] = (xt - xth.astype(np.float32)).astype(ml_dtypes.bfloat16)
            m["wh"] = wh
            m["wl"] = wl
        in_maps.append(m)

    run_kwargs = {}
    if os.environ.get("MEMBRANE_TRACE_DIR"):
        run_kwargs["tmpdir"] = os.environ["MEMBRANE_TRACE_DIR"]
    res = bass_utils.run_bass_kernel_spmd(
        nc, in_maps, core_ids=list(range(NCORES)), **run_kwargs
    )
    LAST_RESULT = res

    syn_full = np.empty((B, T, D), dtype=np.float32)
    mem_full = np.empty((B, T, D), dtype=np.float32)
    for c in range(NCORES):
        r = res.results[c]
        syn_full[c * BC : (c + 1) * BC] = (
            r["syn"].reshape(D, BC, T).transpose(1, 2, 0)
        )
        mem_full[c * BC : (c + 1) * BC] = (
            (r["mem"] * omb_col).reshape(D, BC, T).transpose(1, 2, 0)
        )
    syn_full[:, 0, :] = 0.0
    mem_full[:, 0, :] = 0.0
    return (syn_full, mem_full)
